# revision 1
# baseline (speedup 1.0000x reference)
"""Trn2 Bass kernel for nn_Attention_16793322128104.

Sharding: 8 cores = 2 batches x 4 head-groups (4 heads each).
Each core: QKV projection for its 768 Wqkv columns, 4 attention heads
(softmax with exact per-query max, folded into the S^T matmul as a 65th
contraction row), AV with ones-column denominator, partial out-projection.
Host sums the 4 head-group partials per batch and adds bout.
"""

import sys
from contextlib import ExitStack

import numpy as np

sys.path.insert(0, "/opt/trn_rl_repo")

import concourse.bass as bass
import concourse.bacc as bacc
import concourse.mybir as mybir
from concourse import tile
from concourse.bass_utils import run_bass_kernel_spmd

F32 = mybir.dt.float32
F32R = mybir.dt.float32r
F16 = mybir.dt.float16

N_TOK = 2048          # tokens per batch
DIM = 1024            # model dim
NH = 4                # heads per core
DH = 64               # head dim
SCALE = 8.0           # sqrt(DH); reference MULTIPLIES by sqrt(d_head)

_CACHE = {}


def r32(ap):
    return ap.bitcast(F32R)


def build_nc():
    nc = bacc.Bacc()
    xt_d = nc.declare_dram_parameter("xt", [DIM + 1, N_TOK], F32R, isOutput=False)
    wg_d = nc.declare_dram_parameter("wg", [DIM + 1, 3 * NH * DH], F32R, isOutput=False)
    wout_d = nc.declare_dram_parameter("wout", [NH * DH, DIM], F32R, isOutput=False)
    id_d = nc.declare_dram_parameter("ident", [128, 128], F32, isOutput=False)
    out_d = nc.declare_dram_parameter("out", [DIM, N_TOK], F32, isOutput=True)

    with ExitStack() as ctx:
        tc = ctx.enter_context(tile.TileContext(nc))
        # ---------------- persistent pools ----------------
        qk_pool = ctx.enter_context(tc.tile_pool(name="qk", bufs=1))
        v_pool = ctx.enter_context(tc.tile_pool(name="v", bufs=1))
        misc_pool = ctx.enter_context(tc.tile_pool(name="misc", bufs=1))
        o2_pool = ctx.enter_context(tc.tile_pool(name="o2", bufs=1))
        psum = ctx.enter_context(
            tc.tile_pool(name="psum", bufs=2, space=bass.MemorySpace.PSUM)
        )

        # q2/k2: per-head [65, 2048]: rows 0:64 features, row 64 = shift/ones
        q2 = [qk_pool.tile([DH + 1, N_TOK], F32R, tag=f"q2{h}", name=f"q2{h}") for h in range(NH)]
        k2 = [qk_pool.tile([DH + 1, N_TOK], F32R, tag=f"k2{h}", name=f"k2{h}") for h in range(NH)]
        # v: per key-tile [128, NH, 65] fp16 (col 64 = ones -> denominator)
        vsb = [v_pool.tile([128, NH, DH + 1], F16, tag=f"v{m}", name=f"v{m}") for m in range(16)]
        ident = misc_pool.tile([128, 128], F32, tag="ident", name="identsb")
        ones1 = misc_pool.tile([1, DH], F32R, tag="ones1", name="ones1")
        negmax = [misc_pool.tile([16, 128], F32R, tag=f"nm{h}", name=f"nm{h}") for h in range(NH)]
        o2 = [o2_pool.tile([128, N_TOK], F32R, tag=f"o2{t}", name=f"o2t{t}") for t in range(2)]

        nc.sync.dma_start(ident[:], id_d[:])
        nc.sync.dma_start(ones1[:], xt_d[DIM : DIM + 1, 0:DH])
        for h in range(NH):
            nc.sync.dma_start(k2[h][DH : DH + 1, :], xt_d[DIM : DIM + 1, :])
        for m in range(16):
            nc.vector.memset(vsb[m][:, :, DH : DH + 1], 1.0)

        # ---------------- phase A: QKV projection ----------------
        with (
            tc.tile_pool(name="xt", bufs=1) as xt_pool,
            tc.tile_pool(name="wgp", bufs=1) as wg_pool,
        ):
            xt_all = xt_pool.tile([128, 8, N_TOK], F32R, tag="xta", name="xta")
            xt_row = xt_pool.tile([1, N_TOK], F32R, tag="xt8", name="xt8")
            wg_all = wg_pool.tile([128, 8, 768], F32R, tag="wga", name="wga")
            wg_row = wg_pool.tile([1, 768], F32R, tag="wg8", name="wg8")
            nc.sync.dma_start(
                xt_all[:], xt_d[0:DIM, :].rearrange("(ct p) t -> p ct t", p=128)
            )
            nc.sync.dma_start(xt_row[:], xt_d[DIM : DIM + 1, :])
            nc.sync.dma_start(
                wg_all[:], wg_d[0:DIM, :].rearrange("(ct p) t -> p ct t", p=128)
            )
            nc.sync.dma_start(wg_row[:], wg_d[DIM : DIM + 1, :])
            xt_sb = [xt_all[:, c, :] for c in range(8)] + [xt_row[:]]
            wg_sb = [wg_all[:, c, :] for c in range(8)] + [wg_row[:]]

            # q,k feature-major: [128 f, 512 t] tiles; ft 0,1 -> q; 2,3 -> k
            for ft in range(4):
                col0 = ft * 128 if ft < 2 else 256 + (ft - 2) * 128
                for tj in range(4):
                    ps = psum.tile([128, 512], F32, tag="mm", name="ps")
                    for c in range(9):
                        nc.tensor.matmul(
                            ps[:],
                            wg_sb[c][:, col0 : col0 + 128],
                            xt_sb[c][:, tj * 512 : (tj + 1) * 512],
                            start=(c == 0),
                            stop=(c == 8),
                        )
                    dst = q2 if ft < 2 else k2
                    hb = 2 * (ft % 2)
                    ts = slice(tj * 512, (tj + 1) * 512)
                    nc.scalar.copy(dst[hb][0:DH, ts], ps[0:DH, :])
                    nc.scalar.copy(dst[hb + 1][0:DH, ts], ps[DH:128, :])

            # v token-major: [128 t, 256 f] tiles
            for tt in range(16):
                ps = psum.tile([128, 512], F32, tag="mm", name="ps")
                for c in range(9):
                    nc.tensor.matmul(
                        ps[:, 0:256],
                        xt_sb[c][:, tt * 128 : (tt + 1) * 128],
                        wg_sb[c][:, 512:768],
                        start=(c == 0),
                        stop=(c == 8),
                    )
                nc.scalar.copy(
                    vsb[tt][:, :, 0:DH],
                    ps[:, 0:256].rearrange("p (h d) -> p h d", h=NH),
                )

        # ---------------- phase B: attention per head ----------------
        with tc.tile_pool(name="pt", bufs=1) as pt_pool, tc.tile_pool(
            name="rp", bufs=1
        ) as r_pool, tc.tile_pool(name="mc", bufs=2) as mc_pool:
            PT = pt_pool.tile([128, 16, N_TOK], F16, tag="PT", name="PT")
            for h in range(NH):
                # pass 1: S in [q, k] orientation -> exact row max
                mc = mc_pool.tile([128, 16], F32, tag="mc", name="mc")
                for qt in range(16):
                    ps = psum.tile([128, N_TOK], F32, tag="mm", name="ps")
                    for kc in range(4):
                        nc.tensor.matmul(
                            ps[:, kc * 512 : (kc + 1) * 512],
                            q2[h][0:DH, qt * 128 : (qt + 1) * 128],
                            k2[h][0:DH, kc * 512 : (kc + 1) * 512],
                            start=True,
                            stop=True,
                        )
                    nc.vector.reduce_max(
                        mc[:, qt : qt + 1], ps[:], axis=mybir.AxisListType.X
                    )
                # transpose maxes to a row, negate, DMA into q2 row 64
                pst = psum.tile([16, 128], F32, tag="mm", name="pst")
                nc.tensor.transpose(pst[:], mc[:], ident[:])
                nc.vector.tensor_scalar_mul(negmax[h][:], pst[:], -1.0)
                nc.sync.dma_start(q2[h][DH : DH + 1, :], negmax[h][:])

                # pass 2: S^T with shift folded in; exp -> fp16 P^T
                for m in range(16):
                    ps = psum.tile([128, N_TOK], F32, tag="mm", name="ps")
                    for j in range(4):
                        nc.tensor.matmul(
                            ps[:, j * 512 : (j + 1) * 512],
                            k2[h][:, m * 128 : (m + 1) * 128],
                            q2[h][:, j * 512 : (j + 1) * 512],
                            start=True,
                            stop=True,
                        )
                    nc.scalar.activation(
                        PT[:, m, :], ps[:], mybir.ActivationFunctionType.Exp,
                        scale=SCALE,
                    )

                # AV: o^T[d, t] + denominator row
                po = psum.tile([DH + 1, N_TOK], F32, tag="mm", name="po")
                for j in range(4):
                    for m in range(16):
                        nc.tensor.matmul(
                            po[:, j * 512 : (j + 1) * 512],
                            vsb[m][:, h, :],
                            PT[:, m, j * 512 : (j + 1) * 512],
                            start=(m == 0),
                            stop=(m == 15),
                        )
                # normalize: o2 rows = o^T * (1/denom) broadcast via K=1 matmul
                rr0 = r_pool.tile([1, N_TOK], F32, tag="rr0", name="rr0")
                rr = r_pool.tile([1, N_TOK], F32R, tag="rr", name="rr")
                rm = r_pool.tile([DH, N_TOK], F32, tag="rm", name="rm")
                nc.vector.reciprocal(rr0[:], po[DH : DH + 1, :])
                nc.vector.tensor_copy(rr[:], rr0[:])
                pr = psum.tile([DH, N_TOK], F32, tag="mm", name="pr")
                for j in range(4):
                    nc.tensor.matmul(
                        pr[:, j * 512 : (j + 1) * 512],
                        ones1[:],
                        rr[:, j * 512 : (j + 1) * 512],
                        start=True,
                        stop=True,
                    )
                nc.vector.tensor_copy(rm[:], pr[:])
                o2dst = o2[h // 2][DH * (h % 2) : DH * (h % 2) + DH, :]
                nc.vector.tensor_mul(o2dst, po[0:DH, :], rm[:])

        # ---------------- phase C: out projection ----------------
        with tc.tile_pool(name="ob", bufs=3) as ob_pool, tc.tile_pool(
            name="wop", bufs=1
        ) as wo_pool:
            wout_sb = [wo_pool.tile([128, DIM], F32R, tag=f"wo{t}", name=f"wo{t}") for t in range(2)]
            for t in range(2):
                nc.sync.dma_start(wout_sb[t][:], wout_d[t * 128 : (t + 1) * 128, :])
            for dc in range(8):
                for j in range(4):
                    ps = psum.tile([128, 512], F32, tag="mm", name="ps")
                    for ht in range(2):
                        nc.tensor.matmul(
                            ps[:],
                            wout_sb[ht][:, dc * 128 : (dc + 1) * 128],
                            o2[ht][:, j * 512 : (j + 1) * 512],
                            start=(ht == 0),
                            stop=(ht == 1),
                        )
                    ob = ob_pool.tile([128, 512], F32, tag="ob", name="ob")
                    nc.vector.tensor_copy(ob[:], ps[:])
                    nc.sync.dma_start(
                        out_d[dc * 128 : (dc + 1) * 128, j * 512 : (j + 1) * 512],
                        ob[:],
                    )
    nc.finalize()
    return nc


def _get_nc():
    if "nc" not in _CACHE:
        _CACHE["nc"] = build_nc()
    return _CACHE["nc"]


def kernel(x, Wqkv, bqkv, Wout, bout):
    x = np.asarray(x, np.float32)
    Wqkv = np.asarray(Wqkv, np.float32)
    bqkv = np.asarray(bqkv, np.float32)
    Wout = np.asarray(Wout, np.float32)
    bout = np.asarray(bout, np.float32)
    B = x.shape[0]
    ident = np.eye(128, dtype=np.float32)
    ones_row = np.ones((1, N_TOK), np.float32)

    in_maps = []
    for c in range(8):
        b, g = c // 4, c % 4
        xt = np.concatenate([np.ascontiguousarray(x[b].T), ones_row], 0)
        cols = []
        bias = []
        for blk in range(3):  # q, k, v column blocks of Wqkv
            s = blk * DIM + g * NH * DH
            cols.append(Wqkv[:, s : s + NH * DH])
            bias.append(bqkv[s : s + NH * DH])
        wg = np.concatenate(
            [np.concatenate(cols, 1), np.concatenate(bias)[None, :]], 0
        )
        wo = np.ascontiguousarray(Wout[g * NH * DH : (g + 1) * NH * DH, :])
        in_maps.append(
            {
                "xt": np.ascontiguousarray(xt),
                "wg": np.ascontiguousarray(wg),
                "wout": wo,
                "ident": ident,
            }
        )

    _CACHE["last_in_maps"] = in_maps
    res = run_bass_kernel_spmd(_get_nc(), in_maps, list(range(8))).results
    out = np.empty((B, N_TOK, DIM), np.float32)
    for b in range(B):
        acc = res[4 * b]["out"].astype(np.float32)
        for g in range(1, 4):
            acc = acc + res[4 * b + g]["out"]
        out[b] = acc.T + bout[None, :]
    return out


if __name__ == "__main__":
    rng = np.random.default_rng(0)
    x = rng.standard_normal((2, N_TOK, DIM), np.float32)
    Wqkv = rng.standard_normal((DIM, 3 * DIM), np.float32) * DIM**-0.5
    bqkv = rng.standard_normal((3 * DIM,), np.float32) * 0.02
    Wout = rng.standard_normal((DIM, DIM), np.float32) * DIM**-0.5
    bout = rng.standard_normal((DIM,), np.float32) * 0.02
    o = kernel(x=x, Wqkv=Wqkv, bqkv=bqkv, Wout=Wout, bout=bout)
    print("kernel ran, out shape", o.shape)

